# revision 10
# baseline (speedup 1.0000x reference)
"""Chamfer loss Trainium2 kernel (data-parallel over batch, 8 NeuronCores).

Problem: x, y (8, 4096, 3) fp32; loss = mean_n [ mean_w min_v ||x_nv - y_nw||
+ mean_v min_w ||x_nv - y_nw|| ] (scalar fp32).

Per core (one batch):
  - Host packs augmented operands AX, AY [13, 4096] bf16 via an
    error-compensated hi/lo split so the PE gram matmul produces
    sq[v,w] = ||x_v||^2 + ||y_w||^2 - 2 x_v.y_w at ~fp32 accuracy while
    streaming at bf16 rate (1 col/cycle).
  - PE: 32 m-blocks x 8 matmuls of [13,128]^T @ [13,512] -> PSUM
    [128, 2048] groups (4 banks, double buffered).
  - ACT (ScalarE): evacuates each PSUM group to SBUF fp16 (the only other
    engine that can read PSUM; runs in parallel with DVE).
  - DVE: row-direction min (min over w per v): fp16 2x-mode fold tree
    per m-block + one batched strided reduce per 4 m-blocks;
    col-direction min (min over v per w): fp16 running min chains into
    two [128, 2048] accumulators.
  - Epilogue: PE-transposes of the col accumulators + strided reduces
    -> per-w mins; relu, ACT sqrt with fused free-dim sum accumulation,
    partition sum via PE ones-matvec, scale by 1/V; DMA scalar out.
  - Host averages the 8 per-core partial losses.
"""

import sys

sys.path.insert(0, "/opt/trn_rl_repo")

from contextlib import ExitStack

import ml_dtypes
import numpy as np

import concourse.bacc as bacc
import concourse.tile as tile
from concourse import mybir
from concourse.bass_utils import run_bass_kernel_spmd

BF16 = ml_dtypes.bfloat16

P = 128
V = 4096
KA = 24  # augmented contraction dim (3-way hi/mid/lo split)
NMM = 512  # matmul moving free dim (one fp32 PSUM bank)
GRP = 2048  # PSUM group (4 banks)
NG = V // GRP  # 2 groups per m-block
MB = V // P  # 32 m-blocks
RB = 4  # m-blocks per batched row-min reduce
SCL = 256.0  # fp16 pre-scale: keeps tiny sq minima out of fp16 subnormals

_cache = {}


def _build_nc():
    F32 = mybir.dt.float32
    F16 = mybir.dt.float16
    mn = mybir.AluOpType.min
    X = mybir.AxisListType.X

    nc = bacc.Bacc("TRN2", target_bir_lowering=False)
    ax_d = nc.declare_dram_parameter("ax", [KA, V], mybir.dt.bfloat16, isOutput=False)
    ay_d = nc.declare_dram_parameter("ay", [KA, V], mybir.dt.bfloat16, isOutput=False)
    idh_d = nc.declare_dram_parameter("identh", [P, P], F16, isOutput=False)
    ones_d = nc.declare_dram_parameter("ones", [P, 1], F32, isOutput=False)
    loss_d = nc.declare_dram_parameter("loss", [1, 1], F32, isOutput=True)

    with tile.TileContext(nc) as tc, ExitStack() as ctx:
        const = ctx.enter_context(tc.tile_pool(name="const", bufs=1))
        accs = ctx.enter_context(tc.tile_pool(name="accs", bufs=1))
        copies = ctx.enter_context(tc.tile_pool(name="copies", bufs=6))
        scratch = ctx.enter_context(tc.tile_pool(name="scratch", bufs=3))

        ax_sb = const.tile([KA, V], mybir.dt.bfloat16)
        ay_sb = const.tile([KA, V], mybir.dt.bfloat16)
        idh_sb = const.tile([P, P], F16)
        ones_sb = const.tile([P, 1], F32)
        nc.sync.dma_start(ax_sb[:], ax_d[:])
        nc.sync.dma_start(ay_sb[:], ay_d[:])
        nc.sync.dma_start(idh_sb[:], idh_d[:])
        nc.sync.dma_start(ones_sb[:], ones_d[:])

        cacc4 = accs.tile([P, V], F16, name="cacc4")
        fold4 = accs.tile([P, RB * (GRP // 8)], F16, name="fold4")
        rowmin = accs.tile([P, MB], F32, name="rowmin")
        colmin = accs.tile([P, MB], F32, name="colmin")

        with tc.tile_pool(name="psum", bufs=2, space="PSUM") as psum:
            for m in range(MB):
                lhsT = ax_sb[:, m * P : (m + 1) * P]
                ct = copies.tile([P, V], F16, name="c4k", tag="c4k")
                for g in range(NG):
                    pst = psum.tile([P, GRP], F32, name=f"ps{g}", tag="ps")
                    for j in range(GRP // NMM):
                        c0 = g * GRP + j * NMM
                        nc.tensor.matmul(
                            pst[:, j * NMM : (j + 1) * NMM],
                            lhsT,
                            ay_sb[:, c0 : c0 + NMM],
                            start=True,
                            stop=True,
                        )
                    nc.scalar.activation(
                        ct[:, g * GRP : (g + 1) * GRP], pst[:],
                        mybir.ActivationFunctionType.Copy, scale=SCL,
                    )

                # col-direction running min (one fp16 2x TT over [P, V])
                if m == 0:
                    nc.vector.tensor_copy(cacc4[:], ct[:])
                else:
                    nc.vector.tensor_tensor(cacc4[:], ct[:], cacc4[:], mn)

                # row-direction fold tree: 4096 -> 2048 -> 1024 -> 512 -> 256
                sc = scratch.tile([P, GRP], F16, name="sc", tag="sc")
                nc.vector.tensor_tensor(sc[:], ct[:, :GRP], ct[:, GRP:], mn)
                nc.vector.tensor_tensor(
                    sc[:, : GRP // 2], sc[:, : GRP // 2], sc[:, GRP // 2 :], mn
                )
                nc.vector.tensor_tensor(
                    sc[:, : GRP // 4], sc[:, : GRP // 4],
                    sc[:, GRP // 4 : GRP // 2], mn,
                )
                r = m % RB
                nc.vector.tensor_tensor(
                    fold4[:, r * (GRP // 8) : (r + 1) * (GRP // 8)],
                    sc[:, : GRP // 8],
                    sc[:, GRP // 8 : GRP // 4],
                    mn,
                )
                if r == RB - 1:
                    nc.vector.tensor_reduce(
                        rowmin[:, m - RB + 1 : m + 1],
                        fold4[:].rearrange("p (a b) -> p a b", a=RB),
                        axis=X,
                        op=mn,
                    )

        # Epilogue: transpose col accumulators -> per-w mins.
        with tc.tile_pool(name="psum_ep", bufs=4, space="PSUM") as psum_ep:
            for q in range(8):
                tp = psum_ep.tile([P, 4 * P], F16, name="tp", tag="tp")
                for k in range(4):
                    b = q * 4 + k
                    nc.tensor.transpose(
                        tp[:, k * P : (k + 1) * P],
                        cacc4[:, b * P : (b + 1) * P],
                        idh_sb[:],
                    )
                nc.vector.tensor_reduce(
                    colmin[:, q * 4 : q * 4 + 4],
                    tp[:].rearrange("p (a b) -> p a b", a=4),
                    axis=X,
                    op=mn,
                )

            # relu (tiny negative sq from roundoff), sqrt + fused sum
            nc.vector.tensor_scalar_max(rowmin[:], rowmin[:], 0.0)
            nc.vector.tensor_scalar_max(colmin[:], colmin[:], 0.0)
            sa = accs.tile([P, 1], F32, name="sa")
            sb_ = accs.tile([P, 1], F32, name="sb_")
            nc.scalar.activation(
                rowmin[:], rowmin[:], mybir.ActivationFunctionType.Sqrt,
                accum_out=sa[:],
            )
            nc.scalar.activation(
                colmin[:], colmin[:], mybir.ActivationFunctionType.Sqrt,
                accum_out=sb_[:],
            )
            stot = accs.tile([P, 1], F32, name="stot")
            nc.vector.tensor_add(stot[:], sa[:], sb_[:])
            fin = psum_ep.tile([1, 1], F32, name="fin")
            nc.tensor.matmul(fin[:], stot[:], ones_sb[:], start=True, stop=True)
            res = accs.tile([1, 1], F32, name="res")
            nc.scalar.mul(res[:], fin[:], 1.0 / (V * float(np.sqrt(SCL))))
            nc.sync.dma_start(loss_d[:], res[:])

    nc.finalize()
    return nc


def _split3(v):
    """3-way bf16 split: v ~= h + m + l with residual ~2^-27 |v|."""
    f32 = np.float32
    h = v.astype(BF16)
    m = (v - h.astype(f32)).astype(BF16)
    l = (v - h.astype(f32) - m.astype(f32)).astype(BF16)
    return h, m, l


def _augment(x, y):
    """x, y: (V, 3) fp32 -> AX, AY [24, V] bf16 3-way-split gram operands.

    sq = x2 + y2 + x.(-2y); products kept: hh, hm, mh, hl, lh, mm
    (magnitude >= ~2^-16); x2/y2 carried as 3 bf16 rows each.
    """
    f32 = np.float32
    yy = (-2.0 * y).astype(f32)
    xh, xm, xl = _split3(x)
    yh, ym, yl = _split3(yy)
    x2 = np.einsum("vc,vc->v", x.astype(np.float64), x.astype(np.float64)).astype(f32)
    y2 = np.einsum("vc,vc->v", y.astype(np.float64), y.astype(np.float64)).astype(f32)
    x2h, x2m, x2l = _split3(x2)
    y2h, y2m, y2l = _split3(y2)
    one = np.ones(V, dtype=BF16)

    def cols(a):
        return [a[:, 0], a[:, 1], a[:, 2]]

    ax = np.stack(
        cols(xh) + cols(xh) + cols(xm) + cols(xh) + cols(xl) + cols(xm)
        + [x2h, x2m, x2l, one, one, one]
    )
    ay = np.stack(
        cols(yh) + cols(ym) + cols(yh) + cols(yl) + cols(yh) + cols(ym)
        + [one, one, one, y2h, y2m, y2l]
    )
    return ax, ay


def kernel(x, y):
    x = np.asarray(x, dtype=np.float32)
    y = np.asarray(y, dtype=np.float32)
    n = x.shape[0]
    assert x.shape == (n, V, 3) and y.shape == (n, V, 3) and n == 8

    if "nc" not in _cache:
        _cache["nc"] = _build_nc()
    nc = _cache["nc"]

    identh = np.eye(P, dtype=np.float16)
    ones = np.ones((P, 1), dtype=np.float32)
    in_maps = []
    for i in range(n):
        ax, ay = _augment(x[i], y[i])
        in_maps.append({"ax": ax, "ay": ay, "identh": identh, "ones": ones})

    res = run_bass_kernel_spmd(
        nc, in_maps, list(range(n)), trace=_cache.get("trace", False)
    )
    _cache["last"] = res
    vals = [
        np.asarray(res.results[i]["loss"], dtype=np.float32).reshape(())
        for i in range(n)
    ]
    return np.float32(np.mean(vals))


# revision 11
# speedup vs baseline: 1.0084x; 1.0084x over previous
"""Chamfer loss Trainium2 kernel (data-parallel over batch, 8 NeuronCores).

Problem: x, y (8, 4096, 3) fp32; loss = mean_n [ mean_w min_v ||x_nv - y_nw||
+ mean_v min_w ||x_nv - y_nw|| ] (scalar fp32).

Per core (one batch):
  - Host packs augmented operands AX, AY [13, 4096] bf16 via an
    error-compensated hi/lo split so the PE gram matmul produces
    sq[v,w] = ||x_v||^2 + ||y_w||^2 - 2 x_v.y_w at ~fp32 accuracy while
    streaming at bf16 rate (1 col/cycle).
  - PE: 32 m-blocks x 8 matmuls of [13,128]^T @ [13,512] -> PSUM
    [128, 2048] groups (4 banks, double buffered).
  - ACT (ScalarE): evacuates each PSUM group to SBUF fp16 (the only other
    engine that can read PSUM; runs in parallel with DVE).
  - DVE: row-direction min (min over w per v): fp16 2x-mode fold tree
    per m-block + one batched strided reduce per 4 m-blocks;
    col-direction min (min over v per w): fp16 running min chains into
    two [128, 2048] accumulators.
  - Epilogue: PE-transposes of the col accumulators + strided reduces
    -> per-w mins; relu, ACT sqrt with fused free-dim sum accumulation,
    partition sum via PE ones-matvec, scale by 1/V; DMA scalar out.
  - Host averages the 8 per-core partial losses.
"""

import sys

sys.path.insert(0, "/opt/trn_rl_repo")

from contextlib import ExitStack

import ml_dtypes
import numpy as np

import concourse.bacc as bacc
import concourse.tile as tile
from concourse import mybir
from concourse.bass_utils import run_bass_kernel_spmd

BF16 = ml_dtypes.bfloat16

P = 128
V = 4096
KA = 24  # augmented contraction dim (3-way hi/mid/lo split)
NMM = 512  # matmul moving free dim (one fp32 PSUM bank)
GRP = 2048  # PSUM group (4 banks)
NG = V // GRP  # 2 groups per m-block
MB = V // P  # 32 m-blocks
RB = 4  # m-blocks per batched row-min reduce
SCL = 256.0  # fp16 pre-scale: keeps tiny sq minima out of fp16 subnormals

_cache = {}


def _build_nc():
    F32 = mybir.dt.float32
    F16 = mybir.dt.float16
    mn = mybir.AluOpType.min
    X = mybir.AxisListType.X

    nc = bacc.Bacc("TRN2", target_bir_lowering=False)
    ax_d = nc.declare_dram_parameter("ax", [KA, V], mybir.dt.bfloat16, isOutput=False)
    ay_d = nc.declare_dram_parameter("ay", [KA, V], mybir.dt.bfloat16, isOutput=False)
    idh_d = nc.declare_dram_parameter("identh", [P, P], F16, isOutput=False)
    ones_d = nc.declare_dram_parameter("ones", [P, 1], F32, isOutput=False)
    loss_d = nc.declare_dram_parameter("loss", [1, 1], F32, isOutput=True)

    with tile.TileContext(nc) as tc, ExitStack() as ctx:
        const = ctx.enter_context(tc.tile_pool(name="const", bufs=1))
        accs = ctx.enter_context(tc.tile_pool(name="accs", bufs=1))
        copies = ctx.enter_context(tc.tile_pool(name="copies", bufs=6))
        scratch = ctx.enter_context(tc.tile_pool(name="scratch", bufs=3))

        ax_sb = const.tile([KA, V], mybir.dt.bfloat16)
        ay_sb = const.tile([KA, V], mybir.dt.bfloat16)
        idh_sb = const.tile([P, P], F16)
        ones_sb = const.tile([P, 1], F32)
        CH = V // 4
        for c in range(4):
            nc.sync.dma_start(ax_sb[:, c * CH : (c + 1) * CH], ax_d[:, c * CH : (c + 1) * CH])
            nc.sync.dma_start(ay_sb[:, c * CH : (c + 1) * CH], ay_d[:, c * CH : (c + 1) * CH])
        nc.sync.dma_start(idh_sb[:], idh_d[:])
        nc.sync.dma_start(ones_sb[:], ones_d[:])

        cacc4 = accs.tile([P, V], F16, name="cacc4")
        fold4 = accs.tile([P, RB * (GRP // 8)], F16, name="fold4")
        mins = accs.tile([P, 2 * MB], F32, name="mins")
        rowmin = mins[:, :MB]
        colmin = mins[:, MB:]
        # preload the sqrt ACT table set early so the epilogue pays no load
        warm = accs.tile([1, 1], F32, name="warm")
        nc.scalar.activation(warm[:], ones_sb[0:1, 0:1], mybir.ActivationFunctionType.Sqrt)

        with tc.tile_pool(name="psum", bufs=2, space="PSUM") as psum:
            for m in range(MB):
                lhsT = ax_sb[:, m * P : (m + 1) * P]
                ct = copies.tile([P, V], F16, name="c4k", tag="c4k")
                for g in range(NG):
                    pst = psum.tile([P, GRP], F32, name=f"ps{g}", tag="ps")
                    for j in range(GRP // NMM):
                        c0 = g * GRP + j * NMM
                        nc.tensor.matmul(
                            pst[:, j * NMM : (j + 1) * NMM],
                            lhsT,
                            ay_sb[:, c0 : c0 + NMM],
                            start=True,
                            stop=True,
                        )
                    nc.scalar.activation(
                        ct[:, g * GRP : (g + 1) * GRP], pst[:],
                        mybir.ActivationFunctionType.Copy, scale=SCL,
                    )

                # col-direction running min (one fp16 2x TT over [P, V])
                if m == 0:
                    nc.vector.tensor_copy(cacc4[:], ct[:])
                else:
                    nc.vector.tensor_tensor(cacc4[:], ct[:], cacc4[:], mn)

                # row-direction fold tree: 4096 -> 2048 -> 1024 -> 512 -> 256
                sc = scratch.tile([P, GRP], F16, name="sc", tag="sc")
                nc.vector.tensor_tensor(sc[:], ct[:, :GRP], ct[:, GRP:], mn)
                nc.vector.tensor_tensor(
                    sc[:, : GRP // 2], sc[:, : GRP // 2], sc[:, GRP // 2 :], mn
                )
                nc.vector.tensor_tensor(
                    sc[:, : GRP // 4], sc[:, : GRP // 4],
                    sc[:, GRP // 4 : GRP // 2], mn,
                )
                r = m % RB
                nc.vector.tensor_tensor(
                    fold4[:, r * (GRP // 8) : (r + 1) * (GRP // 8)],
                    sc[:, : GRP // 8],
                    sc[:, GRP // 8 : GRP // 4],
                    mn,
                )
                if r == RB - 1:
                    nc.vector.tensor_reduce(
                        rowmin[:, m - RB + 1 : m + 1],
                        fold4[:].rearrange("p (a b) -> p a b", a=RB),
                        axis=X,
                        op=mn,
                    )

        # Epilogue: transpose col accumulators -> per-w mins.
        with tc.tile_pool(name="psum_ep", bufs=4, space="PSUM") as psum_ep:
            for q in range(8):
                tp = psum_ep.tile([P, 4 * P], F16, name="tp", tag="tp")
                for k in range(4):
                    b = q * 4 + k
                    nc.tensor.transpose(
                        tp[:, k * P : (k + 1) * P],
                        cacc4[:, b * P : (b + 1) * P],
                        idh_sb[:],
                    )
                nc.vector.tensor_reduce(
                    colmin[:, q * 4 : q * 4 + 4],
                    tp[:].rearrange("p (a b) -> p a b", a=4),
                    axis=X,
                    op=mn,
                )

            # relu (tiny negative sq from roundoff), sqrt + fused sum
            nc.vector.tensor_scalar_max(mins[:], mins[:], 0.0)
            stot = accs.tile([P, 1], F32, name="stot")
            nc.scalar.activation(
                mins[:], mins[:], mybir.ActivationFunctionType.Sqrt,
                accum_out=stot[:],
            )
            fin = psum_ep.tile([1, 1], F32, name="fin")
            nc.tensor.matmul(fin[:], stot[:], ones_sb[:], start=True, stop=True)
            res = accs.tile([1, 1], F32, name="res")
            nc.scalar.mul(res[:], fin[:], 1.0 / (V * float(np.sqrt(SCL))))
            nc.sync.dma_start(loss_d[:], res[:])

    nc.finalize()
    return nc


def _split3(v):
    """3-way bf16 split: v ~= h + m + l with residual ~2^-27 |v|."""
    f32 = np.float32
    h = v.astype(BF16)
    m = (v - h.astype(f32)).astype(BF16)
    l = (v - h.astype(f32) - m.astype(f32)).astype(BF16)
    return h, m, l


def _augment(x, y):
    """x, y: (V, 3) fp32 -> AX, AY [24, V] bf16 3-way-split gram operands.

    sq = x2 + y2 + x.(-2y); products kept: hh, hm, mh, hl, lh, mm
    (magnitude >= ~2^-16); x2/y2 carried as 3 bf16 rows each.
    """
    f32 = np.float32
    yy = (-2.0 * y).astype(f32)
    xh, xm, xl = _split3(x)
    yh, ym, yl = _split3(yy)
    x2 = np.einsum("vc,vc->v", x.astype(np.float64), x.astype(np.float64)).astype(f32)
    y2 = np.einsum("vc,vc->v", y.astype(np.float64), y.astype(np.float64)).astype(f32)
    x2h, x2m, x2l = _split3(x2)
    y2h, y2m, y2l = _split3(y2)
    one = np.ones(V, dtype=BF16)

    def cols(a):
        return [a[:, 0], a[:, 1], a[:, 2]]

    ax = np.stack(
        cols(xh) + cols(xh) + cols(xm) + cols(xh) + cols(xl) + cols(xm)
        + [x2h, x2m, x2l, one, one, one]
    )
    ay = np.stack(
        cols(yh) + cols(ym) + cols(yh) + cols(yl) + cols(yh) + cols(ym)
        + [one, one, one, y2h, y2m, y2l]
    )
    return ax, ay


def kernel(x, y):
    x = np.asarray(x, dtype=np.float32)
    y = np.asarray(y, dtype=np.float32)
    n = x.shape[0]
    assert x.shape == (n, V, 3) and y.shape == (n, V, 3) and n == 8

    if "nc" not in _cache:
        _cache["nc"] = _build_nc()
    nc = _cache["nc"]

    identh = np.eye(P, dtype=np.float16)
    ones = np.ones((P, 1), dtype=np.float32)
    in_maps = []
    for i in range(n):
        ax, ay = _augment(x[i], y[i])
        in_maps.append({"ax": ax, "ay": ay, "identh": identh, "ones": ones})

    res = run_bass_kernel_spmd(
        nc, in_maps, list(range(n)), trace=_cache.get("trace", False)
    )
    _cache["last"] = res
    vals = [
        np.asarray(res.results[i]["loss"], dtype=np.float32).reshape(())
        for i in range(n)
    ]
    return np.float32(np.mean(vals))
